# revision 16
# baseline (speedup 1.0000x reference)
"""Trainium2 Bass kernel for CausalSemanticGrouping (slot attention variant).

Reference computation (per batch b):
    slots   = slot_embed                            # [K, D] shared across batch
    sk      = concat([slots, key_b])                # [C=K+N, D]
    dots    = l2norm(slots) @ l2norm(sk).T          # [K, C]  (output 2)
    attn    = softmax(mask(dots) / TEMP)            # strict-lower-tri slot mask
    attn_n  = attn / (sum(attn) + EPS)
    out     = attn_n @ sk                           # [K, D]  (output 1)

Sharding: pure data parallel, batch 64 -> 8 cores x 8 batches.

Per-core pipeline (per batch, split into two 512-token halves):
  1. DMA key half [512, D] natural layout (tokens on partitions).
  2. PE transpose-mode (pure data movement) -> raw key^T chunks in PSUM,
     DVE evacuates to SBUF.  MM1 (f32r): dots_raw = s_nT.T @ key^T.
  3. Norms: ACT Square+accum per chunk -> ss; rsqrt as exp(-0.5*ln(ss))
     (Square/Ln/Exp all live in the natural_log_exp_and_others ACT table
     set -- pinned via the activation-tables hook to avoid set thrash).
     rn [128p, 4] is PE-transposed, gathered to a single row [1, 512],
     and broadcast across the K=64 partitions with a rank-1 f32r matmul.
  4. DVE: dots = dots_raw * rn_bcast (normalization), written to SBUF
     and DMA'd out; ACT: E = exp(dots/TEMP) with accum giving Z row-sums.
  5. PE transpose E -> E^T; MM2 (f32r): out_raw = E^T.T @ [slots; key_raw];
     DVE scales rows by 1/((Z_key + Z_ss) * (1 + EPS)).
"""

import numpy as np

B = 64
NCORES = 8
BL = B // NCORES  # 8 batches per core
N = 1024
D = 512
K = 64
C = K + N  # 1088
TEMP = 0.07
EPS = 1e-7

_CACHE: dict = {}


def _build():
    import concourse.bass as bass
    import concourse.bacc as bacc
    import concourse.hw_specs as hw_specs
    from concourse import mybir
    from concourse.tile import TileContext
    from concourse.masks import make_identity

    F32 = mybir.dt.float32
    F32R = mybir.dt.float32r
    AF = mybir.ActivationFunctionType
    ALU = mybir.AluOpType
    AX = mybir.AxisListType

    nc = bacc.Bacc("TRN2")

    key_d = nc.dram_tensor("key", [BL, N, D], F32, kind="ExternalInput")
    slot_d = nc.dram_tensor("slot_embed", [K, D], F32, kind="ExternalInput")
    out_d = nc.dram_tensor("out", [BL, K, D], F32, kind="ExternalOutput")
    dots_d = nc.dram_tensor("dots", [BL, K, C], F32, kind="ExternalOutput")

    NCH = N // 128  # 8 c-chunks per batch
    DCH = D // 128  # 4 d-chunks

    with TileContext(nc) as tc:
        with (
            tc.tile_pool(name="const", bufs=1) as const,
            tc.tile_pool(name="keyp", bufs=4) as keyp,
            tc.tile_pool(name="ktp", bufs=3) as ktp,
            tc.tile_pool(name="workp", bufs=3) as workp,
            tc.tile_pool(name="outp", bufs=3) as outp,
            tc.tile_pool(name="ptp", bufs=2, space="PSUM") as ptp,
            tc.tile_pool(name="pm1p", bufs=2, space="PSUM") as pm1p,
            tc.tile_pool(name="pm2p", bufs=2, space="PSUM") as pm2p,
            tc.tile_pool(name="pep", bufs=2, space="PSUM") as pep,
        ):
            # ---------------- global (batch-independent) ----------------
            ident = const.tile([128, 128], F32)
            make_identity(nc, ident)

            ident_r = const.tile([128, 128], F32R)
            nc.gpsimd.tensor_copy(out=ident_r, in_=ident)

            ones_f = const.tile([1, K], F32)
            nc.vector.memset(ones_f, 1.0)
            ones_r = const.tile([1, K], F32R)
            nc.gpsimd.tensor_copy(out=ones_r, in_=ones_f)

            slots_raw = const.tile([K, D], F32R)
            nc.sync.dma_start(out=slots_raw, in_=slot_d[:, :].bitcast(F32R))

            # normalize slots: s_n = slots / ||slots||_row
            sq_s = const.tile([K, D], F32)
            ss_s = const.tile([K, 1], F32)
            nc.vector.tensor_mul(sq_s, slots_raw.bitcast(F32),
                                 slots_raw.bitcast(F32))
            nc.vector.reduce_sum(out=ss_s, in_=sq_s, axis=AX.X)
            lnss_s = const.tile([K, 1], F32)
            nc.scalar.activation(out=lnss_s, in_=ss_s, func=AF.Ln)
            rs_s = const.tile([K, 1], F32)
            nc.scalar.activation(out=rs_s, in_=lnss_s, func=AF.Exp, scale=-0.5)
            s_n = const.tile([K, D], F32)
            nc.vector.tensor_scalar_mul(out=s_n, in0=slots_raw.bitcast(F32),
                                        scalar1=rs_s)

            # s_nT: [D, K] as [128, DCH, K]
            s_nT = const.tile([128, DCH, K], F32R)
            ps_snt = ptp.tile([128, 512], F32, tag="t_ps")
            for dc in range(DCH):
                nc.tensor.transpose(
                    out=ps_snt[:, dc * K:(dc + 1) * K],
                    in_=s_n[:, dc * 128:(dc + 1) * 128],
                    identity=ident[:K, :K],
                )
            nc.vector.tensor_copy(
                out=s_nT.rearrange("p a b -> p (a b)"),
                in_=ps_snt[:, : DCH * K],
            )

            # dots_ss = s_n @ s_n.T  [K, K]
            ps_ss = pm1p.tile([K, 512], F32, tag="mm1_ps")
            for dc in range(DCH):
                nc.tensor.matmul(
                    ps_ss[:, :K],
                    lhsT=s_nT[:, dc, :],
                    rhs=s_nT[:, dc, :],
                    start=(dc == 0), stop=(dc == DCH - 1),
                )
            dots_ss = const.tile([K, K], F32)
            nc.vector.tensor_copy(out=dots_ss, in_=ps_ss[:, :K])

            # E_ss = exp(dots_ss / T) masked to strict lower triangle
            e_ss_full = const.tile([K, K], F32)
            nc.scalar.activation(out=e_ss_full, in_=dots_ss, func=AF.Exp,
                                 scale=1.0 / TEMP)
            e_ss = const.tile([K, K], F32)
            # keep where (row - col - 1) >= 0, i.e. col < row; else 0
            nc.gpsimd.affine_select(
                out=e_ss, in_=e_ss_full, compare_op=ALU.is_ge, fill=0.0,
                base=-1, pattern=[[-1, K]], channel_multiplier=1,
            )
            z_ss = const.tile([K, 1], F32)
            nc.vector.reduce_sum(out=z_ss, in_=e_ss, axis=AX.X)

            # write dots_ss into every batch's output block once (bcast AP)
            dss_ap = dots_d[:, :, 0:K]
            dss_src = bass.AP(
                tensor=dots_ss.tensor,
                offset=dots_ss.offset,
                ap=[dots_ss.ap[0], [0, BL], dots_ss.ap[1]],
            )
            nc.gpsimd.dma_start(
                out=dss_ap.rearrange("b k c -> k b c"), in_=dss_src,
            )

            # E_ss^T
            ps_esst = pep.tile([128, 512], F32, tag="e_ps")
            nc.tensor.transpose(out=ps_esst[:K, :K], in_=e_ss,
                                identity=ident[:K, :K])
            e_ssT = const.tile([K, K], F32R)
            nc.vector.tensor_copy(out=e_ssT, in_=ps_esst[:K, :K])

            # ---------------- per-batch ----------------
            for b in range(BL):
                key_sb = keyp.tile([128, NCH, D], F32R, tag="key")
                ss = workp.tile([128, NCH], F32, tag="ss")
                sq = workp.tile([128, D], F32, tag="sq")
                lnss = workp.tile([128, NCH], F32, tag="lnss")
                rn = workp.tile([128, NCH], F32, tag="rn")
                rnT = workp.tile([4, 128], F32, tag="rnT")
                rn_row = workp.tile([1, N], F32R, tag="rn_row")
                keyT = ktp.tile([128, DCH, N], F32R, tag="keyT")
                dots_key = workp.tile([K, N], F32, tag="dots_key")
                e_key = workp.tile([K, N], F32R, tag="e_key")
                z_key = workp.tile([K, 2], F32, tag="z_key")
                ekT = workp.tile([128, NCH, K], F32R, tag="ekT")
                pm2 = pm2p.tile([K, D], F32, tag="mm2_ps")

                # MM2 slot-block: only needs globals; fire early
                nc.tensor.matmul(
                    pm2, lhsT=e_ssT, rhs=slots_raw, start=True, stop=False,
                )

                for h in range(2):
                    hs = slice(4 * h, 4 * h + 4)
                    # load this half of the keys [512, D]
                    nc.sync.dma_start(
                        out=key_sb[:, hs, :],
                        in_=key_d[b, 512 * h:512 * (h + 1)]
                            .rearrange("(t p) d -> p t d", p=128).bitcast(F32R),
                    )

                    # ---- norm chain (parallel to transposes) ----
                    for j in range(4):
                        ci = 4 * h + j
                        nc.scalar.activation(
                            out=sq, in_=key_sb[:, ci, :].bitcast(F32),
                            func=AF.Square, accum_out=ss[:, ci:ci + 1],
                        )
                    nc.scalar.activation(out=lnss[:, hs], in_=ss[:, hs],
                                         func=AF.Ln)
                    nc.scalar.activation(out=rn[:, hs], in_=lnss[:, hs],
                                         func=AF.Exp, scale=-0.5)
                    # rn [128, 4] -> row layout [1, 512] via PE transpose + DMA
                    prt = ptp.tile([128, 512], F32, tag="t_ps")
                    nc.tensor.transpose(out=prt[:4, :128], in_=rn[:, hs],
                                        identity=ident)
                    nc.vector.tensor_copy(out=rnT, in_=prt[:4, :128])
                    nc.gpsimd.dma_start(
                        out=rn_row[0:1, 512 * h:512 * (h + 1)],
                        in_=rnT.bitcast(F32R),
                    )
                    # broadcast rn over the K slot partitions (rank-1 matmul)
                    pb = pep.tile([K, 512], F32, tag="e_ps")
                    nc.tensor.matmul(
                        pb, lhsT=ones_r,
                        rhs=rn_row[0:1, 512 * h:512 * (h + 1)],
                        start=True, stop=True,
                    )
                    rnb = workp.tile([K, 512], F32, tag="rnb")
                    nc.vector.tensor_copy(out=rnb, in_=pb)

                    # ---- raw key^T via PE transpose (pure movement) ----
                    for dc in range(DCH):
                        pt = ptp.tile([128, 512], F32R, tag="t_ps")
                        for j in range(4):
                            ci = 4 * h + j
                            nc.tensor.transpose(
                                out=pt[:, 128 * j:128 * (j + 1)],
                                in_=key_sb[:, ci, 128 * dc:128 * (dc + 1)],
                                identity=ident_r,
                            )
                        if dc == 0:
                            nc.scalar.copy(
                                out=keyT[:, dc, 512 * h:512 * (h + 1)],
                                in_=pt.bitcast(F32),
                            )
                        else:
                            nc.vector.tensor_copy(
                                out=keyT[:, dc, 512 * h:512 * (h + 1)],
                                in_=pt.bitcast(F32),
                            )

                    # MM1: dots_raw[:, half] = s_n @ key^T
                    pm1 = pm1p.tile([K, 512], F32, tag="mm1_ps")
                    for dc in range(DCH):
                        nc.tensor.matmul(
                            pm1,
                            lhsT=s_nT[:, dc, :],
                            rhs=keyT[:, dc, 512 * h:512 * (h + 1)],
                            start=(dc == 0), stop=(dc == DCH - 1),
                        )
                    # normalize: dots = dots_raw * rn (broadcast over rows)
                    nc.vector.tensor_mul(
                        dots_key[:, 512 * h:512 * (h + 1)], pm1, rnb,
                    )
                    nc.gpsimd.dma_start(
                        out=dots_d[b, :, K + 512 * h:K + 512 * (h + 1)],
                        in_=dots_key[:, 512 * h:512 * (h + 1)],
                    )
                    # E = exp(dots/T) with Z row-sums via accumulate
                    nc.scalar.activation(
                        out=e_key[:, 512 * h:512 * (h + 1)],
                        in_=dots_key[:, 512 * h:512 * (h + 1)],
                        func=AF.Exp, scale=1.0 / TEMP,
                        accum_out=z_key[:, h:h + 1],
                    )

                    # E^T for this half + MM2 accumulation
                    pe = pep.tile([128, 256], F32R, tag="e_ps")
                    for j in range(4):
                        nc.tensor.transpose(
                            out=pe[:, K * j:K * (j + 1)],
                            in_=e_key[:, 512 * h + 128 * j:
                                      512 * h + 128 * (j + 1)],
                            identity=ident_r[:K, :K],
                        )
                    nc.vector.tensor_copy(
                        out=ekT[:, hs, :].rearrange("p a b -> p (a b)"),
                        in_=pe.bitcast(F32),
                    )
                    for j in range(4):
                        ck = 4 * h + j
                        nc.tensor.matmul(
                            pm2,
                            lhsT=ekT[:, ck, :],
                            rhs=key_sb[:, ck, :],
                            start=False, stop=(ck == NCH - 1),
                        )

                # scale rows by 1/((z0 + z1 + z_ss) * (1 + EPS))
                zt = workp.tile([K, 1], F32, tag="zt")
                nc.vector.tensor_scalar(
                    out=zt, in0=z_key[:, 0:1], scalar1=z_key[:, 1:2],
                    scalar2=z_ss, op0=ALU.add, op1=ALU.add,
                )
                r = workp.tile([K, 1], F32, tag="r")
                nc.vector.reciprocal(out=r, in_=zt)
                out_sb = outp.tile([K, D], F32, tag="out_sb")
                nc.vector.tensor_scalar(
                    out=out_sb, in0=pm2, scalar1=r, scalar2=1.0 / (1.0 + EPS),
                    op0=ALU.mult, op1=ALU.mult,
                )
                nc.gpsimd.dma_start(out=out_d[b], in_=out_sb)

    # Pin every activation to the one table set containing square+ln+exp so
    # Bacc's table-load pass never inserts per-batch set switches.
    target = "natural_log_exp_and_others"
    orig_tables = hw_specs.get_activation_tables(nc.m.arch)
    assert target in orig_tables
    pinned = {k: (v if k == target else set()) for k, v in orig_tables.items()}
    saved = bacc.get_activation_tables
    bacc.get_activation_tables = lambda arch: pinned
    try:
        nc.finalize()
    finally:
        bacc.get_activation_tables = saved
    return nc


def kernel(key, slot_embed):
    from concourse.bass_utils import run_bass_kernel_spmd

    key = np.ascontiguousarray(np.asarray(key, dtype=np.float32))
    slot_embed = np.ascontiguousarray(np.asarray(slot_embed, dtype=np.float32))
    assert key.shape == (B, N, D) and slot_embed.shape == (K, D)

    if "nc" not in _CACHE:
        _CACHE["nc"] = _build()
    nc = _CACHE["nc"]

    in_maps = [
        {"key": key[i * BL:(i + 1) * BL], "slot_embed": slot_embed}
        for i in range(NCORES)
    ]
    res = run_bass_kernel_spmd(nc, in_maps, core_ids=list(range(NCORES)))
    out = np.concatenate([r["out"] for r in res.results], axis=0)
    dots = np.concatenate([r["dots"] for r in res.results], axis=0)
    return out, dots
